# revision 1
# baseline (speedup 1.0000x reference)
"""GATv2 layer kernel for Trainium2, sharded across 8 NeuronCores.

Computation (reference):
    Wh = h @ W.T                       [N, F]
    s1 = Wh @ a1, s2 = Wh @ a2         [N]
    e  = leaky_relu(s1[:,None] + s2[None,:], 0.2)
    attention = softmax(e * adj, dim=1)
    out = attention @ Wh               [N, F]

Sharding: rows (destination nodes) split across 8 cores, 1024 rows each.
Each core gets its adj row-block plus replicated h/W/a, computes its
1024x128 output block; host concatenates.

Per-core pipeline, tiled [128 rows x 2048 cols]:
    ACT : L = Prelu(SJbc + s1_row, alpha=0.2)      (per-partition bias)
    DVE : T = L * adj_tile
    PE  : transpose T 128x128 tiles -> PSUM
    ACT : P^T = Exp(T^T)  PSUM -> SBUF             (fused evacuation)
    PE  : acc += P^T.T @ [Wh | 1]                  (ones col = softmax denom)
    DVE : out_rows = acc[:, :128] * 1/acc[:, 128]
Softmax is computed without max subtraction: scores are O(6) so exp is
safely in fp32 range, matching the reference up to fp rounding.
"""
import sys

for _p in ("/opt/trn_rl_repo", "/root/.axon_site/_ro/trn_rl_repo"):
    if _p not in sys.path:
        sys.path.insert(0, _p)

import numpy as np
from contextlib import ExitStack

from concourse import bacc, tile, mybir
from concourse.bass_utils import run_bass_kernel_spmd
from concourse.masks import make_identity

f32 = mybir.dt.float32
AL = mybir.AluOpType
AF = mybir.ActivationFunctionType

N = 8192
F = 128
NCORES = 8
RPC = N // NCORES          # rows per core = 1024
RT = RPC // 128            # row tiles per core = 8
CCH = 2048                 # column chunk
NCH = N // CCH             # col chunks per row tile = 4
SUB = CCH // 512           # 512-subchunks per chunk = 4
NEG_SLOPE = 0.2

_CACHE = {}


def _build():
    nc = bacc.Bacc("TRN2", target_bir_lowering=False)

    adj_ext = nc.declare_dram_parameter("adj", [RPC, N], f32, isOutput=False)
    h_ext = nc.declare_dram_parameter("h", [N, F], f32, isOutput=False)
    hloc_ext = nc.declare_dram_parameter("h_loc", [RPC, F], f32, isOutput=False)
    wt_ext = nc.declare_dram_parameter("wt", [F, F], f32, isOutput=False)  # W^T [fi, fo]
    a1_ext = nc.declare_dram_parameter("a1", [F, 1], f32, isOutput=False)
    a2_ext = nc.declare_dram_parameter("a2", [F, 1], f32, isOutput=False)
    out_ext = nc.declare_dram_parameter("out", [RPC, F], f32, isOutput=True)

    with tile.TileContext(nc) as tc, ExitStack() as ctx:
        const = ctx.enter_context(tc.tile_pool(name="const", bufs=1))
        setup = ctx.enter_context(tc.tile_pool(name="setup", bufs=3))
        psum = ctx.enter_context(tc.tile_pool(name="psum", bufs=1, space="PSUM"))
        adj_pool = ctx.enter_context(tc.tile_pool(name="adjp", bufs=3))
        work = ctx.enter_context(tc.tile_pool(name="work", bufs=2))
        pexp = ctx.enter_context(tc.tile_pool(name="pexp", bufs=3))
        outp = ctx.enter_context(tc.tile_pool(name="outp", bufs=2))

        ident = const.tile([128, 128], f32)
        make_identity(nc, ident)
        wt_sb = const.tile([F, F], f32)
        nc.sync.dma_start(out=wt_sb, in_=wt_ext[:, :])
        a1_sb = const.tile([F, 1], f32)
        nc.sync.dma_start(out=a1_sb, in_=a1_ext[:, :])
        a2_sb = const.tile([F, 1], f32)
        nc.sync.dma_start(out=a2_sb, in_=a2_ext[:, :])
        ones_row = const.tile([1, 128], f32)
        nc.vector.memset(ones_row, 1.0)

        # big persistent tensors
        sjbc = const.tile([128, N], f32)              # s2 broadcast over partitions
        whext = const.tile([128, N // 128, F + 1], f32)  # [Wh | 1] tiles, c-partition
        sj_sb = const.tile([1, N], f32)               # s2, free layout
        si_cols = const.tile([128, RT], f32)          # s1 own rows, per-partition cols

        nc.vector.memset(whext[:, :, F:F + 1], 1.0)

        # ---- setup: stream h tiles; build Wh tiles, WhT chunks, s2 ----
        for j in range(N // 128):
            h_t = setup.tile([128, F], f32, tag="h_t")
            nc.sync.dma_start(out=h_t, in_=h_ext[128 * j:128 * j + 128, :])
            ps_hT = psum.tile([128, 512], f32, tag="tp")
            nc.tensor.transpose(ps_hT[:, 0:128], h_t, ident)
            hT_t = setup.tile([128, F], f32, tag="hT_t")  # [fi, n-slice]
            nc.vector.tensor_copy(out=hT_t, in_=ps_hT[:, 0:128])
            # Wh tile [n-slice, fo] = hT_t.T @ WT
            ps_wh = psum.tile([128, F + 1], f32, tag="acc")
            nc.tensor.matmul(ps_wh[:, 0:F], lhsT=hT_t, rhs=wt_sb,
                             start=True, stop=True)
            nc.scalar.copy(out=whext[:, j, 0:F], in_=ps_wh[:, 0:F])
            # WhT chunk [fo, n-slice] = WT.T @ hT_t
            ps_whT = psum.tile([128, 512], f32, tag="tp2")
            nc.tensor.matmul(ps_whT[:, 0:128], lhsT=wt_sb, rhs=hT_t,
                             start=True, stop=True)
            whT_t = setup.tile([128, 128], f32, tag="whT_t")
            nc.vector.tensor_copy(out=whT_t, in_=ps_whT[:, 0:128])
            # s2 chunk [1, n-slice] = a2.T @ WhT_chunk
            ps_sj = psum.tile([1, 512], f32, tag="sj")
            nc.tensor.matmul(ps_sj[:, 0:128], lhsT=a2_sb, rhs=whT_t,
                             start=True, stop=True)
            nc.vector.tensor_copy(out=sj_sb[0:1, 128 * j:128 * j + 128],
                                  in_=ps_sj[0:1, 0:128])

        # ---- own-row s1 column vectors ----
        for t in range(RT):
            hl_t = setup.tile([128, F], f32, tag="h_t")
            nc.sync.dma_start(out=hl_t, in_=hloc_ext[128 * t:128 * t + 128, :])
            ps_hT = psum.tile([128, 512], f32, tag="tp")
            nc.tensor.transpose(ps_hT[:, 0:128], hl_t, ident)
            hTl_t = setup.tile([128, F], f32, tag="hT_t")
            nc.vector.tensor_copy(out=hTl_t, in_=ps_hT[:, 0:128])
            ps_whT = psum.tile([128, 512], f32, tag="tp2")
            nc.tensor.matmul(ps_whT[:, 0:128], lhsT=wt_sb, rhs=hTl_t,
                             start=True, stop=True)
            whTl_t = setup.tile([128, 128], f32, tag="whT_t")
            nc.vector.tensor_copy(out=whTl_t, in_=ps_whT[:, 0:128])
            ps_si = psum.tile([128, F + 1], f32, tag="acc")
            nc.tensor.matmul(ps_si[:, 0:1], lhsT=whTl_t, rhs=a1_sb,
                             start=True, stop=True)
            nc.vector.tensor_copy(out=si_cols[:, t:t + 1], in_=ps_si[:, 0:1])

        # ---- broadcast s2 across partitions ----
        for j in range(N // 512):
            ps_b = psum.tile([128, 512], f32, tag="tp")
            nc.tensor.matmul(ps_b, lhsT=ones_row,
                             rhs=sj_sb[0:1, 512 * j:512 * j + 512],
                             start=True, stop=True)
            nc.scalar.copy(out=sjbc[:, 512 * j:512 * j + 512], in_=ps_b)

        # ---- main loop ----
        for t in range(RT):
            acc = psum.tile([128, F + 1], f32, tag="acc")
            for j in range(NCH):
                adj_t = adj_pool.tile([128, CCH], f32, tag="adj")
                nc.sync.dma_start(
                    out=adj_t,
                    in_=adj_ext[128 * t:128 * t + 128, CCH * j:CCH * j + CCH])
                L = work.tile([128, CCH], f32, tag="L")
                nc.scalar.activation(out=L, in_=sjbc[:, CCH * j:CCH * j + CCH],
                                     func=AF.Prelu, bias=si_cols[:, t:t + 1],
                                     alpha=NEG_SLOPE)
                T = work.tile([128, CCH], f32, tag="T")
                nc.vector.tensor_tensor(out=T, in0=L, in1=adj_t, op=AL.mult)
                for q in range(SUB):
                    tp = psum.tile([128, 512], f32, tag="tp")
                    for s in range(4):
                        nc.tensor.transpose(
                            tp[:, 128 * s:128 * s + 128],
                            T[:, 512 * q + 128 * s:512 * q + 128 * s + 128],
                            ident)
                    P_t = pexp.tile([128, 512], f32, tag="P")
                    nc.scalar.activation(out=P_t, in_=tp, func=AF.Exp)
                    for s in range(4):
                        ci = (CCH * j + 512 * q + 128 * s) // 128
                        nc.tensor.matmul(
                            acc, lhsT=P_t[:, 128 * s:128 * s + 128],
                            rhs=whext[:, ci, :],
                            start=(j == 0 and q == 0 and s == 0),
                            stop=(j == NCH - 1 and q == SUB - 1 and s == 3))
            rinv = outp.tile([128, 1], f32, tag="rinv")
            nc.vector.reciprocal(rinv, acc[:, F:F + 1])
            o_t = outp.tile([128, F], f32, tag="o")
            nc.vector.tensor_scalar(out=o_t, in0=acc[:, 0:F],
                                    scalar1=rinv[:, 0:1], scalar2=None,
                                    op0=AL.mult)
            nc.sync.dma_start(out=out_ext[128 * t:128 * t + 128, :], in_=o_t)

    nc.compile()
    return nc


def _get_nc():
    if "nc" not in _CACHE:
        _CACHE["nc"] = _build()
    return _CACHE["nc"]


def kernel(h, adj, W, a, _trace=False, _trace_kwargs=None):
    h = np.ascontiguousarray(np.asarray(h, dtype=np.float32))
    adj = np.ascontiguousarray(np.asarray(adj, dtype=np.float32))
    W = np.asarray(W, dtype=np.float32)
    a = np.asarray(a, dtype=np.float32)

    wt = np.ascontiguousarray(W.T)                    # [fi, fo]
    a1c = np.ascontiguousarray(a[0, :F].reshape(F, 1))
    a2c = np.ascontiguousarray(a[0, F:].reshape(F, 1))

    nc = _get_nc()
    in_maps = []
    for c in range(NCORES):
        r0 = c * RPC
        in_maps.append({
            "adj": np.ascontiguousarray(adj[r0:r0 + RPC, :]),
            "h": h,
            "h_loc": np.ascontiguousarray(h[r0:r0 + RPC, :]),
            "wt": wt,
            "a1": a1c,
            "a2": a2c,
        })
    kw = {}
    if _trace:
        kw["trace"] = True
        kw.update(_trace_kwargs or {})
    res = run_bass_kernel_spmd(nc, in_maps, core_ids=list(range(NCORES)), **kw)
    out = np.concatenate([res.results[c]["out"] for c in range(NCORES)], axis=0)
    if _trace:
        return out, res
    return out


# revision 2
# speedup vs baseline: 1.4349x; 1.4349x over previous
"""GATv2 layer kernel for Trainium2, sharded across 8 NeuronCores.

Computation (reference):
    Wh = h @ W.T                       [N, F]
    s1 = Wh @ a1, s2 = Wh @ a2         [N]
    e  = leaky_relu(s1[:,None] + s2[None,:], 0.2)
    attention = softmax(e * adj, dim=1)
    out = attention @ Wh               [N, F]

Sharding: rows (destination nodes) split across 8 cores, 1024 rows each.
Each core gets its adj row-block plus replicated h/W/a, computes its
1024x128 output block; host concatenates.

Per-core pipeline, tiled [128 rows x 2048 cols]:
    ACT : L = Prelu(SJbc + s1_row, alpha=0.2)      (per-partition bias)
    DVE : T = L * adj_tile
    PE  : transpose T 128x128 tiles -> PSUM [128,1024]
    ACT : P^T = Exp(T^T)  PSUM -> SBUF bf16        (fused evacuation)
    PE  : acc += P^T.T @ [Wh | 1] (bf16, FWL)      (ones col = softmax denom)
    DVE : out_rows = acc[:, :128] * 1/acc[:, 128]
Softmax is computed without max subtraction: scores are O(6) so exp is
safely in fp32 range, matching the reference up to fp rounding.
"""
import sys

for _p in ("/opt/trn_rl_repo", "/root/.axon_site/_ro/trn_rl_repo"):
    if _p not in sys.path:
        sys.path.insert(0, _p)

import numpy as np
from contextlib import ExitStack

from concourse import bacc, tile, mybir
from concourse.bass_utils import run_bass_kernel_spmd
from concourse.masks import make_identity

f32 = mybir.dt.float32
bf16 = mybir.dt.bfloat16
AL = mybir.AluOpType
AF = mybir.ActivationFunctionType

N = 8192
F = 128
NCORES = 8
RPC = N // NCORES          # rows per core = 1024
RT = RPC // 128            # row tiles per core = 8
CCH = 2048                 # column chunk for PRELU/MULT
NCH = N // CCH             # col chunks per row tile = 4
PSW = 1024                 # psum transpose tile width (2 banks)
NEG_SLOPE = 0.2

_CACHE = {}


def _build():
    nc = bacc.Bacc("TRN2", target_bir_lowering=False)

    adj_ext = nc.declare_dram_parameter("adj", [RPC, N], f32, isOutput=False)
    h_ext = nc.declare_dram_parameter("h", [N, F], f32, isOutput=False)
    hloc_ext = nc.declare_dram_parameter("h_loc", [RPC, F], f32, isOutput=False)
    wt_ext = nc.declare_dram_parameter("wt", [F, F], f32, isOutput=False)  # W^T [fi, fo]
    a1_ext = nc.declare_dram_parameter("a1", [F, 1], f32, isOutput=False)
    a2_ext = nc.declare_dram_parameter("a2", [F, 1], f32, isOutput=False)
    out_ext = nc.declare_dram_parameter("out", [RPC, F], f32, isOutput=True)

    with tile.TileContext(nc) as tc, ExitStack() as ctx:
        const = ctx.enter_context(tc.tile_pool(name="const", bufs=1))
        setup = ctx.enter_context(tc.tile_pool(name="setup", bufs=3))
        ps_tp = ctx.enter_context(tc.tile_pool(name="ps_tp", bufs=2, space="PSUM"))
        ps_acc = ctx.enter_context(tc.tile_pool(name="ps_acc", bufs=2, space="PSUM"))
        adj_pool = ctx.enter_context(tc.tile_pool(name="adjp", bufs=3))
        work = ctx.enter_context(tc.tile_pool(name="work", bufs=2))
        pexp = ctx.enter_context(tc.tile_pool(name="pexp", bufs=3))
        outp = ctx.enter_context(tc.tile_pool(name="outp", bufs=2))

        ident = const.tile([128, 128], f32)
        make_identity(nc, ident)
        wt_sb = const.tile([F, F], f32)
        nc.sync.dma_start(out=wt_sb, in_=wt_ext[:, :])
        a1_sb = const.tile([F, 1], f32)
        nc.sync.dma_start(out=a1_sb, in_=a1_ext[:, :])
        a2_sb = const.tile([F, 1], f32)
        nc.sync.dma_start(out=a2_sb, in_=a2_ext[:, :])
        ones_row = const.tile([1, 128], f32)
        nc.vector.memset(ones_row, 1.0)

        # big persistent tensors
        sjbc = const.tile([128, N], f32)                 # s2 broadcast over partitions
        whext = const.tile([128, N // 128, F + 1], bf16)  # [Wh | 1] tiles, c-partition
        sj_sb = const.tile([1, N], f32)                  # s2, free layout
        si_cols = const.tile([128, RT], f32)             # s1 own rows, per-partition

        nc.vector.memset(whext[:, :, F:F + 1], 1.0)

        # ---- setup: stream h tiles; build Wh tiles, WhT chunks, s2 ----
        for j in range(N // 128):
            h_t = setup.tile([128, F], f32, tag="h_t")
            nc.sync.dma_start(out=h_t, in_=h_ext[128 * j:128 * j + 128, :])
            ps1 = ps_tp.tile([128, PSW], f32, tag="tp")
            nc.tensor.transpose(ps1[:, 0:128], h_t, ident)
            hT_t = setup.tile([128, F], f32, tag="hT_t")  # [fi, n-slice]
            nc.vector.tensor_copy(out=hT_t, in_=ps1[:, 0:128])
            # Wh tile [n-slice, fo] = hT_t.T @ WT ; WhT chunk = WT.T @ hT_t
            ps2 = ps_tp.tile([128, PSW], f32, tag="tp")
            nc.tensor.matmul(ps2[:, 0:F], lhsT=hT_t, rhs=wt_sb,
                             start=True, stop=True)
            nc.tensor.matmul(ps2[:, F:2 * F], lhsT=wt_sb, rhs=hT_t,
                             start=True, stop=True)
            nc.scalar.copy(out=whext[:, j, 0:F], in_=ps2[:, 0:F])
            whT_t = setup.tile([128, 128], f32, tag="whT_t")
            nc.vector.tensor_copy(out=whT_t, in_=ps2[:, F:2 * F])
            # s2 chunk [1, n-slice] = a2.T @ WhT_chunk
            ps3 = ps_acc.tile([128, F + 1], f32, tag="acc")
            nc.tensor.matmul(ps3[0:1, 0:128], lhsT=a2_sb, rhs=whT_t,
                             start=True, stop=True)
            nc.vector.tensor_copy(out=sj_sb[0:1, 128 * j:128 * j + 128],
                                  in_=ps3[0:1, 0:128])

        # ---- own-row s1 column vectors ----
        for t in range(RT):
            hl_t = setup.tile([128, F], f32, tag="h_t")
            nc.sync.dma_start(out=hl_t, in_=hloc_ext[128 * t:128 * t + 128, :])
            ps1 = ps_tp.tile([128, PSW], f32, tag="tp")
            nc.tensor.transpose(ps1[:, 0:128], hl_t, ident)
            hTl_t = setup.tile([128, F], f32, tag="hT_t")
            nc.vector.tensor_copy(out=hTl_t, in_=ps1[:, 0:128])
            ps2 = ps_tp.tile([128, PSW], f32, tag="tp")
            nc.tensor.matmul(ps2[:, 0:128], lhsT=wt_sb, rhs=hTl_t,
                             start=True, stop=True)
            whTl_t = setup.tile([128, 128], f32, tag="whT_t")
            nc.vector.tensor_copy(out=whTl_t, in_=ps2[:, 0:128])
            ps3 = ps_acc.tile([128, F + 1], f32, tag="acc")
            nc.tensor.matmul(ps3[:, 0:1], lhsT=whTl_t, rhs=a1_sb,
                             start=True, stop=True)
            nc.vector.tensor_copy(out=si_cols[:, t:t + 1], in_=ps3[:, 0:1])

        # ---- broadcast s2 across partitions ----
        for j in range(N // 512):
            ps1 = ps_tp.tile([128, PSW], f32, tag="tp")
            nc.tensor.matmul(ps1[:, 0:512], lhsT=ones_row,
                             rhs=sj_sb[0:1, 512 * j:512 * j + 512],
                             start=True, stop=True)
            nc.scalar.copy(out=sjbc[:, 512 * j:512 * j + 512], in_=ps1[:, 0:512])

        # ---- main loop ----
        for t in range(RT):
            acc = ps_acc.tile([128, F + 1], f32, tag="acc")
            for j in range(NCH):
                adj_t = adj_pool.tile([128, CCH], f32, tag="adj")
                nc.sync.dma_start(
                    out=adj_t,
                    in_=adj_ext[128 * t:128 * t + 128, CCH * j:CCH * j + CCH])
                L = work.tile([128, CCH], f32, tag="L")
                nc.scalar.activation(out=L, in_=sjbc[:, CCH * j:CCH * j + CCH],
                                     func=AF.Prelu, bias=si_cols[:, t:t + 1],
                                     alpha=NEG_SLOPE)
                T = work.tile([128, CCH], f32, tag="T")
                nc.vector.tensor_tensor(out=T, in0=L, in1=adj_t, op=AL.mult)
                for q in range(CCH // PSW):
                    tp = ps_tp.tile([128, PSW], f32, tag="tp")
                    for s in range(PSW // 128):
                        nc.tensor.transpose(
                            tp[:, 128 * s:128 * s + 128],
                            T[:, PSW * q + 128 * s:PSW * q + 128 * s + 128],
                            ident)
                    P_t = pexp.tile([128, PSW], bf16, tag="P")
                    nc.scalar.activation(out=P_t, in_=tp, func=AF.Exp)
                    for s in range(PSW // 128):
                        ci = (CCH * j + PSW * q + 128 * s) // 128
                        nc.tensor.matmul(
                            acc, lhsT=P_t[:, 128 * s:128 * s + 128],
                            rhs=whext[:, ci, :],
                            start=(j == 0 and q == 0 and s == 0),
                            stop=(j == NCH - 1 and q == CCH // PSW - 1
                                  and s == PSW // 128 - 1))
            rinv = outp.tile([128, 1], f32, tag="rinv")
            nc.vector.reciprocal(rinv, acc[:, F:F + 1])
            o_t = outp.tile([128, F], f32, tag="o")
            nc.vector.tensor_scalar(out=o_t, in0=acc[:, 0:F],
                                    scalar1=rinv[:, 0:1], scalar2=None,
                                    op0=AL.mult)
            nc.sync.dma_start(out=out_ext[128 * t:128 * t + 128, :], in_=o_t)

    nc.compile()
    return nc


def _get_nc():
    if "nc" not in _CACHE:
        _CACHE["nc"] = _build()
    return _CACHE["nc"]


def kernel(h, adj, W, a, _trace=False, _trace_kwargs=None):
    h = np.ascontiguousarray(np.asarray(h, dtype=np.float32))
    adj = np.ascontiguousarray(np.asarray(adj, dtype=np.float32))
    W = np.asarray(W, dtype=np.float32)
    a = np.asarray(a, dtype=np.float32)

    wt = np.ascontiguousarray(W.T)                    # [fi, fo]
    a1c = np.ascontiguousarray(a[0, :F].reshape(F, 1))
    a2c = np.ascontiguousarray(a[0, F:].reshape(F, 1))

    nc = _get_nc()
    in_maps = []
    for c in range(NCORES):
        r0 = c * RPC
        in_maps.append({
            "adj": np.ascontiguousarray(adj[r0:r0 + RPC, :]),
            "h": h,
            "h_loc": np.ascontiguousarray(h[r0:r0 + RPC, :]),
            "wt": wt,
            "a1": a1c,
            "a2": a2c,
        })
    kw = {}
    if _trace:
        kw["trace"] = True
        kw.update(_trace_kwargs or {})
    res = run_bass_kernel_spmd(nc, in_maps, core_ids=list(range(NCORES)), **kw)
    out = np.concatenate([res.results[c]["out"] for c in range(NCORES)], axis=0)
    if _trace:
        return out, res
    return out


# revision 5
# speedup vs baseline: 1.9302x; 1.3452x over previous
"""GATv2 layer kernel for Trainium2, sharded across 8 NeuronCores.

Computation (reference):
    Wh = h @ W.T                       [N, F]
    s1 = Wh @ a1, s2 = Wh @ a2         [N]
    e  = leaky_relu(s1[:,None] + s2[None,:], 0.2)
    attention = softmax(e * adj, dim=1)
    out = attention @ Wh               [N, F]

Sharding: rows (destination nodes) split across 8 cores, 1024 rows each.
Each core gets its adj row-block plus replicated h/W/a, computes its
1024x128 output block; host concatenates.

Per-core pipeline, tiled [128 rows x 2048 cols]:
    ACT : L = Prelu(SJbc + s1_row, alpha=0.2)      (per-partition bias)
    DVE : T = L * adj_tile
    PE  : transpose T 128x128 tiles -> PSUM [128,1024]
    ACT : P^T = Exp(T^T)  PSUM -> SBUF bf16        (fused evacuation)
    PE  : acc += P^T.T @ [Wh | 1] (bf16, FWL)      (ones col = softmax denom)
    DVE : out_rows = acc[:, :128] * 1/acc[:, 128]
Softmax is computed without max subtraction: scores are O(6) so exp is
safely in fp32 range, matching the reference up to fp rounding.
"""
import sys

for _p in ("/opt/trn_rl_repo", "/root/.axon_site/_ro/trn_rl_repo"):
    if _p not in sys.path:
        sys.path.insert(0, _p)

import numpy as np
from contextlib import ExitStack

from concourse import bacc, tile, mybir
from concourse.bass_utils import run_bass_kernel_spmd
from concourse.masks import make_identity

f32 = mybir.dt.float32
bf16 = mybir.dt.bfloat16
AL = mybir.AluOpType
AF = mybir.ActivationFunctionType

N = 8192
F = 128
NCORES = 8
RPC = N // NCORES          # rows per core = 1024
RT = RPC // 128            # row tiles per core = 8
CCH = 2048                 # column chunk for PRELU/MULT
NCH = N // CCH             # col chunks per row tile = 4
PSW = 1024                 # psum transpose tile width (2 banks)
NEG_SLOPE = 0.2

_CACHE = {}


def _build():
    nc = bacc.Bacc("TRN2", target_bir_lowering=False)

    adj_ext = nc.declare_dram_parameter("adj", [RPC, N], f32, isOutput=False)
    hT_ext = nc.declare_dram_parameter("hT", [F, N], f32, isOutput=False)
    hTloc_ext = nc.declare_dram_parameter("hT_loc", [F, RPC], f32, isOutput=False)
    wt_ext = nc.declare_dram_parameter("wt", [F, F], f32, isOutput=False)  # W^T [fi, fo]
    a1_ext = nc.declare_dram_parameter("a1", [F, 1], f32, isOutput=False)
    a2_ext = nc.declare_dram_parameter("a2", [F, 1], f32, isOutput=False)
    out_ext = nc.declare_dram_parameter("out", [RPC, F], f32, isOutput=True)

    with tile.TileContext(nc) as tc, ExitStack() as ctx:
        const = ctx.enter_context(tc.tile_pool(name="const", bufs=1))
        setup = ctx.enter_context(tc.tile_pool(name="setup", bufs=3))
        ps_tp = ctx.enter_context(tc.tile_pool(name="ps_tp", bufs=2, space="PSUM"))
        ps_acc = ctx.enter_context(tc.tile_pool(name="ps_acc", bufs=2, space="PSUM"))
        adj_pool = ctx.enter_context(tc.tile_pool(name="adjp", bufs=3))
        work = ctx.enter_context(tc.tile_pool(name="work", bufs=2))
        pexp = ctx.enter_context(tc.tile_pool(name="pexp", bufs=3))
        outp = ctx.enter_context(tc.tile_pool(name="outp", bufs=2))

        ident = const.tile([128, 128], f32)
        make_identity(nc, ident)
        ident_bf = const.tile([128, 128], bf16)
        make_identity(nc, ident_bf)
        wt_sb = const.tile([F, F], f32)
        nc.sync.dma_start(out=wt_sb, in_=wt_ext[:, :])
        a1_sb = const.tile([F, 1], f32)
        nc.sync.dma_start(out=a1_sb, in_=a1_ext[:, :])
        a2_sb = const.tile([F, 1], f32)
        nc.sync.dma_start(out=a2_sb, in_=a2_ext[:, :])
        ones_row = const.tile([1, 128], f32)
        nc.vector.memset(ones_row, 1.0)

        # big persistent tensors
        sjbc = const.tile([128, N], f32)                 # s2 broadcast over partitions
        whext = const.tile([128, N // 128, F + 1], bf16)  # [Wh | 1] tiles, c-partition
        sj_sb = const.tile([1, N], f32)                  # s2, free layout
        si_cols = const.tile([128, RT], f32)             # s1 own rows, per-partition

        nc.vector.memset(whext[:, :, F:F + 1], 1.0)

        # ---- setup: stream hT chunks; build Wh tiles, WhT chunks, s2 ----
        for k in range(N // 1024):
            hTc = setup.tile([128, 1024], f32, tag="hTc")
            nc.sync.dma_start(out=hTc, in_=hT_ext[:, 1024 * k:1024 * k + 1024])
            for m in range(8):
                ps2 = ps_tp.tile([128, PSW], f32, tag="tp")
                nc.tensor.matmul(ps2[:, 0:F], lhsT=hTc[:, 128 * m:128 * m + 128],
                                 rhs=wt_sb, start=True, stop=True)
                nc.scalar.copy(out=whext[:, 8 * k + m, 0:F], in_=ps2[:, 0:F])
            for m in range(2):
                ps2 = ps_tp.tile([128, PSW], f32, tag="tp")
                nc.tensor.matmul(ps2[:, 0:512], lhsT=wt_sb,
                                 rhs=hTc[:, 512 * m:512 * m + 512],
                                 start=True, stop=True)
                whT_c = setup.tile([128, 512], f32, tag="whT_c")
                nc.vector.tensor_copy(out=whT_c, in_=ps2[:, 0:512])
                ps3 = ps_tp.tile([128, PSW], f32, tag="tp")
                nc.tensor.matmul(ps3[0:1, 0:512], lhsT=a2_sb,
                                 rhs=whT_c[:, 0:512], start=True, stop=True)
                off = 1024 * k + 512 * m
                nc.vector.tensor_copy(out=sj_sb[0:1, off:off + 512],
                                      in_=ps3[0:1, 0:512])

        # ---- own-row s1 column vectors ----
        for k in range(RPC // 512):
            hTlc = setup.tile([128, 512], f32, tag="whT_c")
            nc.sync.dma_start(out=hTlc, in_=hTloc_ext[:, 512 * k:512 * k + 512])
            ps2 = ps_tp.tile([128, PSW], f32, tag="tp")
            nc.tensor.matmul(ps2[:, 0:512], lhsT=wt_sb, rhs=hTlc,
                             start=True, stop=True)
            whTl_c = setup.tile([128, 512], f32, tag="whT_c")
            nc.vector.tensor_copy(out=whTl_c, in_=ps2[:, 0:512])
            for m in range(4):
                t = 4 * k + m
                ps3 = ps_acc.tile([128, F + 1], f32, tag="acc")
                nc.tensor.matmul(ps3[:, 0:1],
                                 lhsT=whTl_c[:, 128 * m:128 * m + 128],
                                 rhs=a1_sb, start=True, stop=True)
                nc.vector.tensor_copy(out=si_cols[:, t:t + 1], in_=ps3[:, 0:1])

        # ---- broadcast s2 across partitions ----
        for j in range(N // 512):
            ps1 = ps_tp.tile([128, PSW], f32, tag="tp")
            nc.tensor.matmul(ps1[:, 0:512], lhsT=ones_row,
                             rhs=sj_sb[0:1, 512 * j:512 * j + 512],
                             start=True, stop=True)
            nc.scalar.copy(out=sjbc[:, 512 * j:512 * j + 512], in_=ps1[:, 0:512])

        # ---- main loop ----
        for t in range(RT):
            acc = ps_acc.tile([128, F + 1], f32, tag="acc")
            for j in range(NCH):
                adj_t = adj_pool.tile([128, CCH], f32, tag="adj")
                nc.sync.dma_start(
                    out=adj_t,
                    in_=adj_ext[128 * t:128 * t + 128, CCH * j:CCH * j + CCH])
                L = work.tile([128, CCH], f32, tag="L")
                nc.scalar.activation(out=L, in_=sjbc[:, CCH * j:CCH * j + CCH],
                                     func=AF.Prelu, bias=si_cols[:, t:t + 1],
                                     alpha=NEG_SLOPE)
                T = work.tile([128, CCH], bf16, tag="T")
                nc.vector.tensor_tensor(out=T, in0=L, in1=adj_t, op=AL.mult)
                for q in range(CCH // PSW):
                    tp = ps_tp.tile([128, PSW], bf16, tag="tp")
                    for s in range(PSW // 128):
                        nc.tensor.transpose(
                            tp[:, 128 * s:128 * s + 128],
                            T[:, PSW * q + 128 * s:PSW * q + 128 * s + 128],
                            ident_bf)
                    P_t = pexp.tile([128, PSW], bf16, tag="P")
                    nc.scalar.activation(out=P_t, in_=tp, func=AF.Exp)
                    for s in range(PSW // 128):
                        ci = (CCH * j + PSW * q + 128 * s) // 128
                        nc.tensor.matmul(
                            acc, lhsT=P_t[:, 128 * s:128 * s + 128],
                            rhs=whext[:, ci, :],
                            start=(j == 0 and q == 0 and s == 0),
                            stop=(j == NCH - 1 and q == CCH // PSW - 1
                                  and s == PSW // 128 - 1))
            rinv = outp.tile([128, 1], f32, tag="rinv")
            nc.vector.reciprocal(rinv, acc[:, F:F + 1])
            o_t = outp.tile([128, F], f32, tag="o")
            nc.vector.tensor_scalar(out=o_t, in0=acc[:, 0:F],
                                    scalar1=rinv[:, 0:1], scalar2=None,
                                    op0=AL.mult)
            nc.sync.dma_start(out=out_ext[128 * t:128 * t + 128, :], in_=o_t)

    nc.compile()
    return nc


def _get_nc():
    if "nc" not in _CACHE:
        _CACHE["nc"] = _build()
    return _CACHE["nc"]


def kernel(h, adj, W, a, _trace=False, _trace_kwargs=None):
    h = np.ascontiguousarray(np.asarray(h, dtype=np.float32))
    adj = np.ascontiguousarray(np.asarray(adj, dtype=np.float32))
    W = np.asarray(W, dtype=np.float32)
    a = np.asarray(a, dtype=np.float32)

    wt = np.ascontiguousarray(W.T)                    # [fi, fo]
    a1c = np.ascontiguousarray(a[0, :F].reshape(F, 1))
    a2c = np.ascontiguousarray(a[0, F:].reshape(F, 1))

    hT = np.ascontiguousarray(h.T)                    # [fi, n]
    nc = _get_nc()
    in_maps = []
    for c in range(NCORES):
        r0 = c * RPC
        in_maps.append({
            "adj": np.ascontiguousarray(adj[r0:r0 + RPC, :]),
            "hT": hT,
            "hT_loc": np.ascontiguousarray(hT[:, r0:r0 + RPC]),
            "wt": wt,
            "a1": a1c,
            "a2": a2c,
        })
    kw = {}
    if _trace:
        kw["trace"] = True
        kw.update(_trace_kwargs or {})
    res = run_bass_kernel_spmd(nc, in_maps, core_ids=list(range(NCORES)), **kw)
    out = np.concatenate([res.results[c]["out"] for c in range(NCORES)], axis=0)
    if _trace:
        return out, res
    return out


# revision 9
# speedup vs baseline: 2.1583x; 1.1182x over previous
"""GATv2 layer kernel for Trainium2, sharded across 8 NeuronCores.

Computation (reference):
    Wh = h @ W.T                       [N, F]
    s1 = Wh @ a1, s2 = Wh @ a2         [N]
    e  = leaky_relu(s1[:,None] + s2[None,:], 0.2)
    attention = softmax(e * adj, dim=1)
    out = attention @ Wh               [N, F]

Sharding: rows (destination nodes) split across 8 cores, 1024 rows each.
Each core gets its adj row-block plus replicated h/W/a, computes its
1024x128 output block; host concatenates.

Per-core pipeline, tiled [128 rows x 2048 cols]:
    ACT : L = Prelu(SJbc + s1_row, alpha=0.2)      (per-partition bias)
    DVE : T = L * adj_tile
    PE  : transpose T 128x128 tiles -> PSUM [128,1024]
    ACT : P^T = Exp(T^T)  PSUM -> SBUF bf16        (fused evacuation)
    PE  : acc += P^T.T @ [Wh | 1] (bf16, FWL)      (ones col = softmax denom)
    DVE : out_rows = acc[:, :128] * 1/acc[:, 128]
Softmax is computed without max subtraction: scores are O(6) so exp is
safely in fp32 range, matching the reference up to fp rounding.
"""
import sys

for _p in ("/opt/trn_rl_repo", "/root/.axon_site/_ro/trn_rl_repo"):
    if _p not in sys.path:
        sys.path.insert(0, _p)

import numpy as np
from contextlib import ExitStack

from concourse import bacc, tile, mybir
from concourse.bass_utils import run_bass_kernel_spmd
from concourse.masks import make_identity

f32 = mybir.dt.float32
bf16 = mybir.dt.bfloat16
AL = mybir.AluOpType
AF = mybir.ActivationFunctionType

N = 8192
F = 128
NCORES = 8
RPC = N // NCORES          # rows per core = 1024
RT = RPC // 128            # row tiles per core = 8
CCH = 2048                 # column chunk for PRELU/MULT
NCH = N // CCH             # col chunks per row tile = 4
PSW = 1024                 # psum transpose tile width (2 banks)
NEG_SLOPE = 0.2

_CACHE = {}


def _build():
    nc = bacc.Bacc("TRN2", target_bir_lowering=False)

    adj_ext = nc.declare_dram_parameter("adj", [RPC, N], f32, isOutput=False)
    hT_ext = nc.declare_dram_parameter("hT", [F, N], f32, isOutput=False)
    hTloc_ext = nc.declare_dram_parameter("hT_loc", [F, RPC], f32, isOutput=False)
    wt_ext = nc.declare_dram_parameter("wt", [F, F], f32, isOutput=False)  # W^T [fi, fo]
    a1_ext = nc.declare_dram_parameter("a1", [F, 1], f32, isOutput=False)
    a2_ext = nc.declare_dram_parameter("a2", [F, 1], f32, isOutput=False)
    out_ext = nc.declare_dram_parameter("out", [RPC, F], f32, isOutput=True)

    with tile.TileContext(nc) as tc, ExitStack() as ctx:
        const = ctx.enter_context(tc.tile_pool(name="const", bufs=1))
        setup = ctx.enter_context(tc.tile_pool(name="setup", bufs=3))
        ps_tp = ctx.enter_context(tc.tile_pool(name="ps_tp", bufs=3, space="PSUM"))
        ps_acc = ctx.enter_context(tc.tile_pool(name="ps_acc", bufs=2, space="PSUM"))
        adj_pool = ctx.enter_context(tc.tile_pool(name="adjp", bufs=4))
        work = ctx.enter_context(tc.tile_pool(name="work", bufs=3))
        pexp = ctx.enter_context(tc.tile_pool(name="pexp", bufs=4))
        outp = ctx.enter_context(tc.tile_pool(name="outp", bufs=2))

        ident = const.tile([128, 128], f32)
        make_identity(nc, ident)
        ident_bf = const.tile([128, 128], bf16)
        make_identity(nc, ident_bf)
        wt_sb = const.tile([F, F], f32)
        nc.sync.dma_start(out=wt_sb, in_=wt_ext[:, :])
        a1_sb = const.tile([F, 1], f32)
        nc.sync.dma_start(out=a1_sb, in_=a1_ext[:, :])
        a2_sb = const.tile([F, 1], f32)
        nc.sync.dma_start(out=a2_sb, in_=a2_ext[:, :])
        ones_row = const.tile([1, 128], f32)
        nc.vector.memset(ones_row, 1.0)

        # big persistent tensors, split per column-chunk so the main loop can
        # start on chunk j as soon as its slice of setup is done
        sjbc_t = [const.tile([128, CCH], f32, name=f"sjbc{_}") for _ in range(NCH)]
        whext_t = [const.tile([128, CCH // 128, F + 1], bf16, name=f"whext{_}") for _ in range(NCH)]
        sj_t = [const.tile([1, CCH], f32, name=f"sj{_}") for _ in range(NCH)]
        si_cols = const.tile([128, RT], f32)             # s1 own rows, per-partition

        for jj in range(NCH):
            nc.vector.memset(whext_t[jj][:, :, F:F + 1], 1.0)

        # ---- own-row s1 column vectors ----
        for k in range(RPC // 512):
            hTlc = setup.tile([128, 512], f32, tag="whT_c")
            nc.sync.dma_start(out=hTlc, in_=hTloc_ext[:, 512 * k:512 * k + 512])
            ps2 = ps_tp.tile([128, PSW], f32, tag="tp")
            nc.tensor.matmul(ps2[:, 0:512], lhsT=wt_sb, rhs=hTlc,
                             start=True, stop=True)
            whTl_c = setup.tile([128, 512], f32, tag="whT_c")
            nc.vector.tensor_copy(out=whTl_c, in_=ps2[:, 0:512])
            for m in range(4):
                t = 4 * k + m
                ps3 = ps_acc.tile([128, F + 1], f32, tag="acc")
                nc.tensor.matmul(ps3[:, 0:1],
                                 lhsT=whTl_c[:, 128 * m:128 * m + 128],
                                 rhs=a1_sb, start=True, stop=True)
                nc.vector.tensor_copy(out=si_cols[:, t:t + 1], in_=ps3[:, 0:1])

        # ---- setup: stream hT chunks; build Wh tiles, WhT chunks, s2 ----
        for k in range(N // 1024):
            hTc = setup.tile([128, 1024], f32, tag="hTc")
            nc.sync.dma_start(out=hTc, in_=hT_ext[:, 1024 * k:1024 * k + 1024])
            for m in range(8):
                ps2 = ps_tp.tile([128, PSW], f32, tag="tp")
                nc.tensor.matmul(ps2[:, 0:F], lhsT=hTc[:, 128 * m:128 * m + 128],
                                 rhs=wt_sb, start=True, stop=True)
                ci = 8 * k + m
                nc.vector.tensor_copy(out=whext_t[ci // 16][:, ci % 16, 0:F],
                                      in_=ps2[:, 0:F])
            for m in range(2):
                ps2 = ps_tp.tile([128, PSW], f32, tag="tp")
                nc.tensor.matmul(ps2[:, 0:512], lhsT=wt_sb,
                                 rhs=hTc[:, 512 * m:512 * m + 512],
                                 start=True, stop=True)
                whT_c = setup.tile([128, 512], f32, tag="whT_c")
                nc.vector.tensor_copy(out=whT_c, in_=ps2[:, 0:512])
                ps3 = ps_tp.tile([128, PSW], f32, tag="tp")
                nc.tensor.matmul(ps3[0:1, 0:512], lhsT=a2_sb,
                                 rhs=whT_c[:, 0:512], start=True, stop=True)
                off = 1024 * k + 512 * m
                nc.vector.tensor_copy(
                    out=sj_t[off // CCH][0:1, off % CCH:off % CCH + 512],
                    in_=ps3[0:1, 0:512])

        # ---- broadcast s2 across partitions (stage via DRAM, then
        # partition-broadcast DMA: zero partition stride needs a DRAM src) ----
        sj_dram = nc.dram_tensor("sj_stage", [NCH, CCH], f32)
        for jj in range(NCH):
            nc.sync.dma_start(out=sj_dram[jj:jj + 1, :], in_=sj_t[jj][0:1, :])
            nc.sync.dma_start(out=sjbc_t[jj],
                              in_=sj_dram[jj:jj + 1, :].to_broadcast([128, CCH]))

        # ---- main loop ----
        for t in range(RT):
            acc = ps_acc.tile([128, F + 1], f32, tag="acc")
            for j in range(NCH):
                adj_t = adj_pool.tile([128, CCH], f32, tag="adj")
                nc.sync.dma_start(
                    out=adj_t,
                    in_=adj_ext[128 * t:128 * t + 128, CCH * j:CCH * j + CCH])
                L = work.tile([128, CCH], f32, tag="L")
                nc.scalar.activation(out=L, in_=sjbc_t[j][:, :],
                                     func=AF.Prelu, bias=si_cols[:, t:t + 1],
                                     alpha=NEG_SLOPE)
                T = work.tile([128, CCH], bf16, tag="T")
                nc.vector.tensor_tensor(out=T, in0=L, in1=adj_t, op=AL.mult)
                for q in range(CCH // PSW):
                    tp = ps_tp.tile([128, PSW], bf16, tag="tp")
                    for s in range(PSW // 128):
                        nc.tensor.transpose(
                            tp[:, 128 * s:128 * s + 128],
                            T[:, PSW * q + 128 * s:PSW * q + 128 * s + 128],
                            ident_bf)
                    P_t = pexp.tile([128, PSW], bf16, tag="P")
                    nc.scalar.activation(out=P_t, in_=tp, func=AF.Exp)
                    for s in range(PSW // 128):
                        ci = (PSW * q + 128 * s) // 128
                        nc.tensor.matmul(
                            acc, lhsT=P_t[:, 128 * s:128 * s + 128],
                            rhs=whext_t[j][:, ci, :],
                            start=(j == 0 and q == 0 and s == 0),
                            stop=(j == NCH - 1 and q == CCH // PSW - 1
                                  and s == PSW // 128 - 1))
            rinv = outp.tile([128, 1], f32, tag="rinv")
            nc.vector.reciprocal(rinv, acc[:, F:F + 1])
            o_t = outp.tile([128, F], f32, tag="o")
            nc.vector.tensor_scalar(out=o_t, in0=acc[:, 0:F],
                                    scalar1=rinv[:, 0:1], scalar2=None,
                                    op0=AL.mult)
            nc.sync.dma_start(out=out_ext[128 * t:128 * t + 128, :], in_=o_t)

    nc.compile()
    return nc


def _get_nc():
    if "nc" not in _CACHE:
        _CACHE["nc"] = _build()
    return _CACHE["nc"]


def kernel(h, adj, W, a, _trace=False, _trace_kwargs=None):
    h = np.ascontiguousarray(np.asarray(h, dtype=np.float32))
    adj = np.ascontiguousarray(np.asarray(adj, dtype=np.float32))
    W = np.asarray(W, dtype=np.float32)
    a = np.asarray(a, dtype=np.float32)

    wt = np.ascontiguousarray(W.T)                    # [fi, fo]
    a1c = np.ascontiguousarray(a[0, :F].reshape(F, 1))
    a2c = np.ascontiguousarray(a[0, F:].reshape(F, 1))

    hT = np.ascontiguousarray(h.T)                    # [fi, n]
    nc = _get_nc()
    in_maps = []
    for c in range(NCORES):
        r0 = c * RPC
        in_maps.append({
            "adj": np.ascontiguousarray(adj[r0:r0 + RPC, :]),
            "hT": hT,
            "hT_loc": np.ascontiguousarray(hT[:, r0:r0 + RPC]),
            "wt": wt,
            "a1": a1c,
            "a2": a2c,
        })
    kw = {}
    if _trace:
        kw["trace"] = True
        kw.update(_trace_kwargs or {})
    res = run_bass_kernel_spmd(nc, in_maps, core_ids=list(range(NCORES)), **kw)
    out = np.concatenate([res.results[c]["out"] for c in range(NCORES)], axis=0)
    if _trace:
        return out, res
    return out


# revision 10
# speedup vs baseline: 2.5421x; 1.1778x over previous
"""GATv2 layer kernel for Trainium2, sharded across 8 NeuronCores.

Computation (reference):
    Wh = h @ W.T                       [N, F]
    s1 = Wh @ a1, s2 = Wh @ a2         [N]
    e  = leaky_relu(s1[:,None] + s2[None,:], 0.2)
    attention = softmax(e * adj, dim=1)
    out = attention @ Wh               [N, F]

Sharding: rows (destination nodes) split across 8 cores, 1024 rows each.
Each core gets its adj row-block plus replicated h/W/a, computes its
1024x128 output block; host concatenates.

Per-core pipeline, tiled [128 rows x 2048 cols]:
    ACT : L = Prelu(SJbc + s1_row, alpha=0.2)      (per-partition bias)
    DVE : T = L * adj_tile
    PE  : transpose T 128x128 tiles -> PSUM [128,1024]
    ACT : P^T = Exp(T^T)  PSUM -> SBUF bf16        (fused evacuation)
    PE  : acc += P^T.T @ [Wh | 1] (bf16, FWL)      (ones col = softmax denom)
    DVE : out_rows = acc[:, :128] * 1/acc[:, 128]
Softmax is computed without max subtraction: scores are O(6) so exp is
safely in fp32 range, matching the reference up to fp rounding.
"""
import sys

for _p in ("/opt/trn_rl_repo", "/root/.axon_site/_ro/trn_rl_repo"):
    if _p not in sys.path:
        sys.path.insert(0, _p)

import numpy as np
from contextlib import ExitStack

from concourse import bacc, tile, mybir
from concourse.bass_utils import run_bass_kernel_spmd
from concourse.masks import make_identity

f32 = mybir.dt.float32
bf16 = mybir.dt.bfloat16
AL = mybir.AluOpType
AF = mybir.ActivationFunctionType

N = 8192
F = 128
NCORES = 8
RPC = N // NCORES          # rows per core = 1024
RT = RPC // 128            # row tiles per core = 8
CCH = 2048                 # column chunk for PRELU/MULT
NCH = N // CCH             # col chunks per row tile = 4
PSW = 1024                 # psum transpose tile width (2 banks)
NEG_SLOPE = 0.2

_CACHE = {}


def _build():
    nc = bacc.Bacc("TRN2", target_bir_lowering=False)

    adj_ext = nc.declare_dram_parameter("adj", [RPC, N], f32, isOutput=False)
    hT_ext = nc.declare_dram_parameter("hT", [F, N], f32, isOutput=False)
    hTloc_ext = nc.declare_dram_parameter("hT_loc", [F, RPC], f32, isOutput=False)
    wt_ext = nc.declare_dram_parameter("wt", [F, F], f32, isOutput=False)  # W^T [fi, fo]
    w_ext = nc.declare_dram_parameter("w", [F, F], f32, isOutput=False)    # W [fo, fi]
    a1_ext = nc.declare_dram_parameter("a1", [F, 1], f32, isOutput=False)
    a2_ext = nc.declare_dram_parameter("a2", [F, 1], f32, isOutput=False)
    out_ext = nc.declare_dram_parameter("out", [RPC, F], f32, isOutput=True)

    with tile.TileContext(nc) as tc, ExitStack() as ctx:
        const = ctx.enter_context(tc.tile_pool(name="const", bufs=1))
        setup = ctx.enter_context(tc.tile_pool(name="setup", bufs=3))
        ps_tp = ctx.enter_context(tc.tile_pool(name="ps_tp", bufs=3, space="PSUM"))
        ps_acc = ctx.enter_context(tc.tile_pool(name="ps_acc", bufs=2, space="PSUM"))
        adj_pool = ctx.enter_context(tc.tile_pool(name="adjp", bufs=4))
        work = ctx.enter_context(tc.tile_pool(name="work", bufs=3))
        pexp = ctx.enter_context(tc.tile_pool(name="pexp", bufs=4))
        outp = ctx.enter_context(tc.tile_pool(name="outp", bufs=2))

        ident = const.tile([128, 128], f32)
        make_identity(nc, ident)
        ident_bf = const.tile([128, 128], bf16)
        make_identity(nc, ident_bf)
        wt_sb = const.tile([F, F], f32)
        nc.sync.dma_start(out=wt_sb, in_=wt_ext[:, :])
        w_sb = const.tile([F, F], f32)
        nc.sync.dma_start(out=w_sb, in_=w_ext[:, :])
        a1_sb = const.tile([F, 1], f32)
        nc.sync.dma_start(out=a1_sb, in_=a1_ext[:, :])
        a2_sb = const.tile([F, 1], f32)
        nc.sync.dma_start(out=a2_sb, in_=a2_ext[:, :])
        ones_row = const.tile([1, 128], f32)
        nc.vector.memset(ones_row, 1.0)

        # big persistent tensors, split per column-chunk so the main loop can
        # start on chunk j as soon as its slice of setup is done
        sjbc_t = [const.tile([128, CCH], f32, name=f"sjbc{_}") for _ in range(NCH)]
        whext_t = [const.tile([128, CCH // 128, F + 1], bf16, name=f"whext{_}") for _ in range(NCH)]
        sj_t = [const.tile([1, CCH], f32, name=f"sj{_}") for _ in range(NCH)]
        si_cols = const.tile([128, RT], f32)             # s1 own rows, per-partition

        for jj in range(NCH):
            nc.vector.memset(whext_t[jj][:, :, F:F + 1], 1.0)

        # ---- w1 = W^T a1, w2 = W^T a2 (feature-space vectors) ----
        ps_w = ps_tp.tile([128, PSW], f32, tag="tp")
        nc.tensor.matmul(ps_w[:, 0:1], lhsT=w_sb, rhs=a1_sb, start=True, stop=True)
        nc.tensor.matmul(ps_w[:, 1:2], lhsT=w_sb, rhs=a2_sb, start=True, stop=True)
        w1c = const.tile([128, 1], f32)
        nc.vector.tensor_copy(out=w1c, in_=ps_w[:, 0:1])
        w2c = const.tile([128, 1], f32)
        nc.vector.tensor_copy(out=w2c, in_=ps_w[:, 1:2])

        # ---- own-row s1 column vectors: si = hTloc^T @ w1 ----
        for kk in range(RPC // 512):
            hTlc = setup.tile([128, 512], f32, tag="whT_c")
            nc.gpsimd.dma_start(out=hTlc,
                                in_=hTloc_ext[:, 512 * kk:512 * kk + 512])
            for m in range(4):
                t = 4 * kk + m
                ps3 = ps_acc.tile([128, F + 1], f32, tag="acc")
                nc.tensor.matmul(ps3[:, 0:1],
                                 lhsT=hTlc[:, 128 * m:128 * m + 128],
                                 rhs=w1c, start=True, stop=True)
                nc.vector.tensor_copy(out=si_cols[:, t:t + 1], in_=ps3[:, 0:1])

        # ---- stream hT chunks; build Wh tiles and s2 = hT^T @ w2 ----
        sj_dram = nc.dram_tensor("sj_stage", [NCH, CCH], f32)
        for k in range(N // 1024):
            hTc = setup.tile([128, 1024], f32, tag="hTc")
            nc.gpsimd.dma_start(out=hTc, in_=hT_ext[:, 1024 * k:1024 * k + 1024])
            for m in range(8):
                ps2 = ps_tp.tile([128, PSW], f32, tag="tp")
                nc.tensor.matmul(ps2[:, 0:F], lhsT=hTc[:, 128 * m:128 * m + 128],
                                 rhs=wt_sb, start=True, stop=True)
                ci = 8 * k + m
                nc.vector.tensor_copy(out=whext_t[ci // 16][:, ci % 16, 0:F],
                                      in_=ps2[:, 0:F])
            for m in range(2):
                ps3 = ps_tp.tile([128, PSW], f32, tag="tp")
                nc.tensor.matmul(ps3[0:1, 0:512], lhsT=w2c,
                                 rhs=hTc[:, 512 * m:512 * m + 512],
                                 start=True, stop=True)
                off = 1024 * k + 512 * m
                nc.vector.tensor_copy(
                    out=sj_t[off // CCH][0:1, off % CCH:off % CCH + 512],
                    in_=ps3[0:1, 0:512])
            if k % 2 == 1:
                jj = k // 2
                nc.gpsimd.dma_start(out=sj_dram[jj:jj + 1, :],
                                    in_=sj_t[jj][0:1, :])
                nc.gpsimd.dma_start(
                    out=sjbc_t[jj],
                    in_=sj_dram[jj:jj + 1, :].to_broadcast([128, CCH]))

        # ---- main loop ----
        for t in range(RT):
            acc = ps_acc.tile([128, F + 1], f32, tag="acc")
            for j in range(NCH):
                adj_t = adj_pool.tile([128, CCH], f32, tag="adj")
                nc.sync.dma_start(
                    out=adj_t,
                    in_=adj_ext[128 * t:128 * t + 128, CCH * j:CCH * j + CCH])
                L = work.tile([128, CCH], f32, tag="L")
                nc.scalar.activation(out=L, in_=sjbc_t[j][:, :],
                                     func=AF.Prelu, bias=si_cols[:, t:t + 1],
                                     alpha=NEG_SLOPE)
                T = work.tile([128, CCH], bf16, tag="T")
                nc.vector.tensor_tensor(out=T, in0=L, in1=adj_t, op=AL.mult)
                for q in range(CCH // PSW):
                    tp = ps_tp.tile([128, PSW], bf16, tag="tp")
                    for s in range(PSW // 128):
                        nc.tensor.transpose(
                            tp[:, 128 * s:128 * s + 128],
                            T[:, PSW * q + 128 * s:PSW * q + 128 * s + 128],
                            ident_bf)
                    P_t = pexp.tile([128, PSW], bf16, tag="P")
                    nc.scalar.activation(out=P_t, in_=tp, func=AF.Exp)
                    for s in range(PSW // 128):
                        ci = (PSW * q + 128 * s) // 128
                        nc.tensor.matmul(
                            acc, lhsT=P_t[:, 128 * s:128 * s + 128],
                            rhs=whext_t[j][:, ci, :],
                            start=(j == 0 and q == 0 and s == 0),
                            stop=(j == NCH - 1 and q == CCH // PSW - 1
                                  and s == PSW // 128 - 1))
            rinv = outp.tile([128, 1], f32, tag="rinv")
            nc.vector.reciprocal(rinv, acc[:, F:F + 1])
            o_t = outp.tile([128, F], f32, tag="o")
            nc.vector.tensor_scalar(out=o_t, in0=acc[:, 0:F],
                                    scalar1=rinv[:, 0:1], scalar2=None,
                                    op0=AL.mult)
            nc.sync.dma_start(out=out_ext[128 * t:128 * t + 128, :], in_=o_t)

    nc.compile()
    return nc


def _get_nc():
    if "nc" not in _CACHE:
        _CACHE["nc"] = _build()
    return _CACHE["nc"]


def kernel(h, adj, W, a, _trace=False, _trace_kwargs=None):
    h = np.ascontiguousarray(np.asarray(h, dtype=np.float32))
    adj = np.ascontiguousarray(np.asarray(adj, dtype=np.float32))
    W = np.asarray(W, dtype=np.float32)
    a = np.asarray(a, dtype=np.float32)

    wt = np.ascontiguousarray(W.T)                    # [fi, fo]
    a1c = np.ascontiguousarray(a[0, :F].reshape(F, 1))
    a2c = np.ascontiguousarray(a[0, F:].reshape(F, 1))

    hT = np.ascontiguousarray(h.T)                    # [fi, n]
    nc = _get_nc()
    in_maps = []
    for c in range(NCORES):
        r0 = c * RPC
        in_maps.append({
            "adj": np.ascontiguousarray(adj[r0:r0 + RPC, :]),
            "hT": hT,
            "hT_loc": np.ascontiguousarray(hT[:, r0:r0 + RPC]),
            "wt": wt,
            "w": W,
            "a1": a1c,
            "a2": a2c,
        })
    kw = {}
    if _trace:
        kw["trace"] = True
        kw.update(_trace_kwargs or {})
    res = run_bass_kernel_spmd(nc, in_maps, core_ids=list(range(NCORES)), **kw)
    out = np.concatenate([res.results[c]["out"] for c in range(NCORES)], axis=0)
    if _trace:
        return out, res
    return out
